# revision 32
# baseline (speedup 1.0000x reference)
"""AttentionSequencePoolingLayer Trainium2 kernel (8-core data parallel).

B=2048, S=200, D=64, H1=64, H2=16. Batch sharded 256/core.
Dataflow per core, per group of 16 batch rows (8 pairs, 2 tokens chunks
128+80):
  kT tiles via SWDGE cast-load + xbar transpose (as before).
  qk^T per pair via DVE tensor_scalar (2x mode).
  x1 = z1 + qW accumulated fully in PSUM: rank-2 one-hot matmul folds the
       per-pair qW broadcast, so dice-1 bias/scale become pair-independent.
  p1/t1/h1 batched over 2 pairs (416 cols) to amortize per-op overhead:
       p1 = sigmoid(inv1*x1 - m1*inv1)   [ACT]
       t1 = p1*(1-a1) + a1               [DVE TS 2x]
       h1 = x1 * t1                      [DVE TT, PSUM read]
  z2 = W2^T h1 — single matmul per pair (alpha folded into t1, not W2).
  p2/t2/h2 batched over all 8 pairs (416 cols).
  scores: block-diagonal W3 -> 4 matmuls per group (token-major PSUM).
  w = sigmoid(scores)*mask; out = w^T k on PE (as before).
"""
import numpy as np
import ml_dtypes

import concourse.bacc as bacc
import concourse.tile as tile
import concourse.mybir as mybir
import concourse.bass as bass
from concourse.bass_utils import run_bass_kernel_spmd

B, S, D = 2048, 200, 64
H1, H2 = 64, 16
EPS = 1e-9
NCORES = 8
BLOC = B // NCORES          # 256 batch rows per core
NGROUPS_FULL = BLOC // 16   # 16

F32 = mybir.dt.float32
BF16 = mybir.dt.bfloat16
AF = mybir.ActivationFunctionType
ALU = mybir.AluOpType
bf = ml_dtypes.bfloat16

LP_BUFS = 6
KT_BUFS = 3
WP_BUFS = 4
PSX_BUFS = 3
PSZ_BUFS = 2
H1P_BUFS = 4
OP_BUFS = 4
GF_BUFS = None
GP_BUFS = None
GORDER = None

_CACHE = {}
TRACE = False
LAST_RESULT = None


def _build(ngroups, pipelined=False, t1_mode=1, po_mode=2, wt_pool=False, t2_act=False, weff_act=0, weff_pre=False, po2=False):
    nc = bacc.Bacc("TRN2", target_bir_lowering=False, debug=False, num_devices=NCORES)
    nb = 16 * ngroups           # batch rows this build processes
    npair = nb // 2

    key = nc.dram_tensor("key", [nb * S, D], F32, kind="ExternalInput").ap()
    qp = nc.dram_tensor("qp", [128, npair], F32, kind="ExternalInput").ap()
    qw2 = nc.dram_tensor("qw2", [2, 64 * npair], BF16, kind="ExternalInput").ap()
    onehot = nc.dram_tensor("onehot", [2, 416], BF16, kind="ExternalInput").ap()
    mask = nc.dram_tensor("mask", [128, 32 * ngroups], BF16, kind="ExternalInput").ap()
    wk2 = nc.dram_tensor("wk2", [128, 128], BF16, kind="ExternalInput").ap()
    wqk2 = nc.dram_tensor("wqk2", [128, 128], BF16, kind="ExternalInput").ap()
    w2b = nc.dram_tensor("w2b", [128, 32], BF16, kind="ExternalInput").ap()
    w34 = nc.dram_tensor("w34", [128, 8], BF16, kind="ExternalInput").ap()
    cols = nc.dram_tensor("cols", [128, 8], F32, kind="ExternalInput").ap()
    # cols: 0=inv1 1=-m1*inv1 2=1-a1 3=a1 4=inv2 5=-m2*inv2 6=1-a2 7=a2
    out = nc.dram_tensor("out", [nb, D], F32, kind="ExternalOutput").ap()

    key_r = key.rearrange("(b s) d -> s b d", s=S)  # [200, nb, 64] view

    with tile.TileContext(nc) as tc:
        with (
            tc.tile_pool(name="const", bufs=1) as cp,
            tc.tile_pool(name="load", bufs=LP_BUFS) as lp,
            tc.tile_pool(name="kt", bufs=KT_BUFS) as ktp,
            tc.tile_pool(name="qk", bufs=KT_BUFS) as qkp,
            tc.tile_pool(name="work", bufs=WP_BUFS) as wp,
            tc.tile_pool(name="h1p", bufs=H1P_BUFS) as h1p,
            tc.tile_pool(name="outp", bufs=OP_BUFS) as op_,
            tc.tile_pool(name="psx", bufs=PSX_BUFS, space="PSUM") as psx,
            tc.tile_pool(name="psz", bufs=PSZ_BUFS, space="PSUM") as psz,
            tc.tile_pool(name="psp", bufs=2, space="PSUM") as psp,
        ):
            # ---- constants into SBUF
            c_qp = cp.tile([128, npair], F32)
            nc.sync.dma_start(out=c_qp[:], in_=qp)
            c_qw2 = cp.tile([2, 64 * npair], BF16)
            nc.sync.dma_start(out=c_qw2[:], in_=qw2)
            c_oh = cp.tile([2, 416], BF16)
            nc.sync.dma_start(out=c_oh[:], in_=onehot)
            c_mask = cp.tile([128, 32 * ngroups], BF16)
            nc.sync.dma_start(out=c_mask[:], in_=mask)
            c_wk = cp.tile([128, 128], BF16)
            nc.sync.dma_start(out=c_wk[:], in_=wk2)
            c_wqk = cp.tile([128, 128], BF16)
            nc.sync.dma_start(out=c_wqk[:], in_=wqk2)
            c_w2 = cp.tile([128, 32], BF16)
            nc.sync.dma_start(out=c_w2[:], in_=w2b)
            c_w34 = cp.tile([128, 8], BF16)
            nc.sync.dma_start(out=c_w34[:], in_=w34)
            c_cols = cp.tile([128, 8], F32)
            nc.sync.dma_start(out=c_cols[:], in_=cols)

            st = {}   # per-group live tiles

            def stage_load(g):
                gf = lp.tile([128, 16, 64], BF16, tag="gf", name=f"gf{g}", bufs=GF_BUFS)
                nc.gpsimd.dma_start(out=gf[:], in_=key_r[0:128, 16 * g : 16 * g + 16, :])
                gp = lp.tile([128, 16, 64], BF16, tag="gp", name=f"gp{g}", bufs=GP_BUFS)
                prow = 72 if g == ngroups - 1 else 80
                if g >= ngroups - 2:
                    nc.vector.memset(gp[64:96, :, :], 0.0)
                nc.gpsimd.dma_start(
                    out=gp[0:prow, :, :],
                    in_=bass.AP(
                        key.tensor,
                        (16 * g * S + 128) * D,
                        [[D, prow], [S * D, 16], [1, D]],
                    ),
                )
                st.setdefault(g, {}).update(gf=gf, gp=gp)

            def stage_kt(g):
                s = st[g]
                gf, gp = s["gf"], s["gp"]
                ktf = ktp.tile([128, 8, 128], BF16, tag="ktf", name=f"ktf{g}")
                nc.sync.dma_start(out=ktf[:], in_=gf.rearrange("p b d -> p (b d)"),
                                  transpose=True)
                ktq = ktp.tile([128, 8, 80], BF16, tag="ktq", name=f"ktq{g}")
                nc.sync.dma_start(
                    out=ktq[:],
                    in_=gp[0:80, :, :].rearrange("p b d -> p (b d)"),
                    transpose=True,
                )
                s.update(ktf=ktf, ktq=ktq)

            def stage_weff(g):
                # wqs[:, jj, :] = diag(q_pair) @ Wqk — depends only on consts,
                # so it prefetches arbitrarily deep (off the critical chain)
                s = st[g]
                wqs = qkp.tile([128, 8, 128], BF16, tag="wqs", name=f"wqs{g}",
                               bufs=ngroups if weff_pre else None)
                for jj in range(8):
                    j = 8 * g + jj
                    if jj < weff_act:
                        nc.scalar.activation(wqs[:, jj, :], c_wqk[:], AF.Copy,
                                             scale=c_qp[:, j : j + 1])
                    else:
                        nc.vector.tensor_scalar(wqs[:, jj, :], c_wqk[:],
                                                c_qp[:, j : j + 1], None, ALU.mult)
                s["wqs"] = wqs

            def stage_a(g):
                # z1 -> x1 PSUM -> p1 -> t1 -> h1 -> z2, per half-quad (2 pairs)
                s = st[g]
                ktf, ktq, wqs = s["ktf"], s["ktq"], s["wqs"]
                s["z2"] = psz.tile([128, 416], F32, tag="z2", name=f"z2_{g}")
                h1s = []
                for m in range(4):
                    x1 = psx.tile([128, 416], F32, tag="x1", name=f"x1_{g}_{m}")
                    hq = 4 * g + m
                    nc.tensor.matmul(x1[:, 0:416],
                                     c_qw2[:, 128 * hq : 128 * hq + 128],
                                     c_oh[:], start=True, stop=False)
                    for k in range(2):
                        jj = 2 * m + k
                        base = 208 * k
                        nc.tensor.matmul(x1[:, base : base + 128], c_wk[:],
                                         ktf[:, jj, :], start=False, stop=False)
                        nc.tensor.matmul(x1[:, base : base + 128], wqs[:, jj, :],
                                         ktf[:, jj, :], start=False, stop=True)
                        nc.tensor.matmul(x1[:, base + 128 : base + 208], c_wk[:],
                                         ktq[:, jj, :], start=False, stop=False)
                        nc.tensor.matmul(x1[:, base + 128 : base + 208], wqs[:, jj, :],
                                         ktq[:, jj, :], start=False, stop=True)
                    p1 = wp.tile([128, 416], BF16, tag="p1", name=f"p1_{g}_{m}")
                    nc.scalar.activation(p1[:], x1[:], AF.Sigmoid,
                                         bias=c_cols[:, 1:2], scale=c_cols[:, 0:1])
                    t1 = wp.tile([128, 416], BF16, tag="t1", name=f"t1_{g}_{m}")
                    use_act = t1_mode == 1 or (t1_mode == 2 and m % 2 == 0)
                    if use_act:
                        nc.scalar.activation(t1[:], p1[:], AF.Identity,
                                             bias=c_cols[:, 3:4], scale=c_cols[:, 2:3])
                    else:
                        nc.vector.tensor_scalar(t1[:], p1[:], c_cols[:, 2:3],
                                                c_cols[:, 3:4], ALU.mult, ALU.add)
                    h1 = h1p.tile([128, 416], BF16, tag="h1", name=f"h1_{g}_{m}")
                    nc.vector.tensor_tensor(h1[:], x1[:], t1[:], ALU.mult)
                    for k in range(2):
                        jj = 2 * m + k
                        q4, jq = jj // 4, jj % 4
                        nc.tensor.matmul(s["z2"][32 * jq : 32 * jq + 32,
                                                 208 * q4 : 208 * q4 + 208],
                                         c_w2[:], h1[:, 208 * k : 208 * k + 208],
                                         start=True, stop=True,
                                         tile_position=(0, 32 * jq))
                    h1s.append(h1)
                s["h1s"] = h1s

            def stage_b1(g):
                # dice2 elementwise (z2 matmuls live in stage_a per half-quad)
                s = st[g]
                z2 = s["z2"]
                p2 = wp.tile([128, 416], BF16, tag="p2", name=f"p2_{g}")
                nc.scalar.activation(p2[:], z2[:], AF.Sigmoid,
                                     bias=c_cols[:, 5:6], scale=c_cols[:, 4:5])
                t2 = wp.tile([128, 416], BF16, tag="t2", name=f"t2_{g}")
                if t2_act:
                    nc.scalar.activation(t2[:], p2[:], AF.Identity,
                                         bias=c_cols[:, 7:8], scale=c_cols[:, 6:7])
                else:
                    nc.vector.tensor_scalar(t2[:], p2[:], c_cols[:, 6:7], c_cols[:, 7:8],
                                            ALU.mult, ALU.add)
                h2 = wp.tile([128, 416], BF16, tag="h2", name=f"h2_{g}")
                nc.vector.tensor_tensor(h2[:], z2[:], t2[:], ALU.mult)
                s["h2"] = h2

            def stage_b2(g):
                # scores -> sigmoid -> mask
                s = st[g]
                h2 = s["h2"]
                scores = psz.tile([128, 32], F32, tag="sc", bufs=1, name=f"sc{g}")
                for q4 in range(2):
                    nc.tensor.matmul(scores[0:128, 8 * q4 : 8 * q4 + 8],
                                     h2[:, 208 * q4 : 208 * q4 + 128], c_w34[:],
                                     start=True, stop=True)
                    nc.tensor.matmul(scores[0:80, 16 + 8 * q4 : 24 + 8 * q4],
                                     h2[:, 208 * q4 + 128 : 208 * q4 + 208], c_w34[:],
                                     start=True, stop=True)
                sg = wp.tile([128, 32], BF16, tag="sg", name=f"sg{g}")
                nc.scalar.activation(sg[:, 0:16], scores[:, 0:16], AF.Sigmoid)
                nc.scalar.activation(sg[0:80, 16:32], scores[0:80, 16:32], AF.Sigmoid)
                wt = wp.tile([128, 32], BF16, tag="wt", name=f"wt{g}")
                nc.vector.tensor_tensor(wt[:, 0:16], sg[:, 0:16],
                                        c_mask[:, 32 * g : 32 * g + 16], ALU.mult)
                nc.vector.tensor_tensor(wt[0:80, 16:32], sg[0:80, 16:32],
                                        c_mask[0:80, 32 * g + 16 : 32 * g + 32],
                                        ALU.mult)
                s["wt"] = wt

            def stage_b3(g):
                # pooling + output
                s = st[g]
                gf, gp, wt = s["gf"], s["gp"], s["wt"]
                if po2:
                    pool = psp.tile([128, 512], F32, tag="pool", name=f"pool{g}")
                else:
                    pool = psp.tile([128, 256], F32, tag="pool", name=f"pool{g}")
                for jw in range(8):
                    if po2:
                        pb = 32 * (jw // 4)
                        po = 128 * (jw % 4)
                    else:
                        pb = 32 * (jw // 2)
                        po = 128 * (jw % 2)
                    rhs_f = gf[:, 2 * jw : 2 * jw + 2, :].rearrange("p b d -> p (b d)")
                    rhs_p = gp[0:80, 2 * jw : 2 * jw + 2, :].rearrange("p b d -> p (b d)")
                    nc.tensor.matmul(pool[pb : pb + 2, po : po + 128],
                                     wt[0:128, 2 * jw : 2 * jw + 2], rhs_f,
                                     start=True, stop=False, tile_position=(0, pb))
                    nc.tensor.matmul(pool[pb : pb + 2, po : po + 128],
                                     wt[0:80, 16 + 2 * jw : 16 + 2 * jw + 2], rhs_p,
                                     start=False, stop=True, tile_position=(0, pb))
                if po2:
                    po_sb = op_.tile([128, 512], F32, tag="po", name=f"po{g}")
                    nc.vector.tensor_copy(po_sb[0:2, :], pool[0:2, :])
                    nc.vector.tensor_copy(po_sb[32:34, :], pool[32:34, :])
                    for bh in range(2):
                        src = po_sb[bh:64:32, :].rearrange("p (ji x) -> p ji x", x=128)
                        src = src[:, :, 64 * bh : 64 * bh + 64]
                        dst = bass.AP(out.tensor, (16 * g + bh) * D,
                                      [[8 * D, 2], [2 * D, 4], [1, D]])
                        nc.sync.dma_start(out=dst, in_=src)
                    del st[g]
                    return
                po_sb = op_.tile([128, 256], F32, tag="po", name=f"po{g}")
                if po_mode == 0:
                    eng = [nc.scalar] * 4
                elif po_mode == 1:
                    eng = [nc.scalar, nc.vector, nc.scalar, nc.vector]
                elif po_mode == 3:
                    eng = [nc.gpsimd] * 4
                elif po_mode == 4:
                    eng = [nc.vector, nc.vector, nc.vector, nc.scalar]
                elif po_mode == 5:
                    eng = [nc.scalar, nc.vector, nc.vector, nc.vector]
                else:
                    eng = [nc.vector] * 4
                for i, e in enumerate(eng):
                    r = 32 * i
                    if e is nc.scalar:
                        e.copy(po_sb[r : r + 2, :], pool[r : r + 2, :])
                    else:
                        e.tensor_copy(po_sb[r : r + 2, :], pool[r : r + 2, :])
                del st[g]
                # out rows: b = 16g + 4*bi + 2*jj + bh; sbuf row 32*bi+bh,
                # col 128*jj + 64*bh + d
                for bh in range(2):
                    src = po_sb[bh:128:32, :].rearrange("p (jj x) -> p jj x", x=128)
                    src = src[:, :, 64 * bh : 64 * bh + 64]
                    dst = bass.AP(out.tensor, (16 * g + bh) * D,
                                  [[4 * D, 4], [2 * D, 2], [1, D]])
                    nc.sync.dma_start(out=dst, in_=src)

            if pipelined:
                # software pipeline: loads 3 ahead, transposes/weff 2 ahead,
                # stage-A 1 ahead
                for gg in range(min(3, ngroups)):
                    stage_load(gg)
                for gg in range(min(2, ngroups)):
                    stage_weff(gg)
                for gg in range(min(2, ngroups)):
                    stage_kt(gg)
                stage_a(0)
                for g in range(ngroups):
                    if g + 3 < ngroups:
                        stage_load(g + 3)
                    stage_b1(g)
                    if g + 2 < ngroups:
                        stage_kt(g + 2)
                    stage_b2(g)
                    if g + 1 < ngroups:
                        stage_a(g + 1)
                    stage_b3(g)
                    if g + 2 < ngroups:
                        stage_weff(g + 2)
            else:
                if weff_pre:
                    for g in range(ngroups):
                        st[g] = {}
                        stage_weff(g)
                order = GORDER if GORDER and len(GORDER) == ngroups else range(ngroups)
                for g in order:
                    if weff_pre:
                        stage_load(g)
                    else:
                        stage_load(g)
                        stage_weff(g)
                    stage_kt(g)
                    stage_a(g)
                    stage_b1(g)
                    stage_b2(g)
                    stage_b3(g)
    nc.compile()
    return nc


def _prep_consts(W1, alpha1, mean1, var1, W2, alpha2, mean2, var2, W3):
    inv1 = 1.0 / np.sqrt(var1 + EPS)
    inv2 = 1.0 / np.sqrt(var2 + EPS)
    Wq = W1[0:64] + W1[128:192]
    Wk = W1[64:128] - W1[128:192]
    Wqk = W1[192:256]

    def blk(a):
        m = np.zeros((128, 2 * a.shape[1]), np.float32)
        m[0:64, 0 : a.shape[1]] = a
        m[64:128, a.shape[1] :] = a
        return m

    wk2 = blk(Wk).astype(bf)
    wqk2 = blk(Wqk).astype(bf)
    w2b = blk(W2).astype(bf)
    # w34: row 32*jq + 16*bh + h -> col 2*jq + bh = W3[h]
    w34 = np.zeros((128, 8), np.float32)
    for jq in range(4):
        for bh in range(2):
            w34[32 * jq + 16 * bh : 32 * jq + 16 * bh + 16, 2 * jq + bh] = W3[:, 0]
    w34 = w34.astype(bf)
    cols = np.zeros((128, 8), np.float32)
    cols[:, 0] = np.tile(inv1, 2)
    cols[:, 1] = np.tile(-mean1 * inv1, 2)
    cols[:, 2] = np.tile(1.0 - alpha1, 2)
    cols[:, 3] = np.tile(alpha1, 2)
    cols[:, 4] = np.tile(inv2, 8)
    cols[:, 5] = np.tile(-mean2 * inv2, 8)
    cols[:, 6] = np.tile(1.0 - alpha2, 8)
    cols[:, 7] = np.tile(alpha2, 8)
    onehot = np.zeros((2, 416), np.float32)
    onehot[0, 0:208] = 1.0
    onehot[1, 208:416] = 1.0
    return Wq, wk2, wqk2, w2b, w34, cols, onehot.astype(bf)


def kernel(query_emb, key_emb, seq_length, W1, alpha1, mean1, var1,
           W2, alpha2, mean2, var2, W3, _ngroups=NGROUPS_FULL):
    (Wq, wk2, wqk2, w2b, w34, cols, onehot) = _prep_consts(
        np.asarray(W1, np.float32), np.asarray(alpha1, np.float32),
        np.asarray(mean1, np.float32), np.asarray(var1, np.float32),
        np.asarray(W2, np.float32), np.asarray(alpha2, np.float32),
        np.asarray(mean2, np.float32), np.asarray(var2, np.float32),
        np.asarray(W3, np.float32))
    q = np.asarray(query_emb, np.float32)
    k = np.asarray(key_emb, np.float32)
    sl = np.asarray(seq_length).reshape(-1)

    if _ngroups not in _CACHE:
        _CACHE[_ngroups] = _build(_ngroups)
    nc = _CACHE[_ngroups]
    nb = 16 * _ngroups
    npair = nb // 2

    qW = q @ Wq  # [B, 64]

    in_maps = []
    for c in range(NCORES):
        b0 = c * BLOC
        qs = q[b0 : b0 + nb]
        qWs = qW[b0 : b0 + nb]
        sls = sl[b0 : b0 + nb]
        # pair layouts [128=(bhat,d), npair]
        qp_t = np.zeros((128, npair), np.float32)
        for bh in range(2):
            qp_t[64 * bh : 64 * bh + 64] = qs[bh::2].T
        # qw2 [2, 64*npair]: row k, cols 128*hq + 64*bh + h = qW[16g+2*(2m+k)+bh, h]
        # hq = 4g+m; pair j = 2*hq + k... j = 8g + 2m + k; batch row = 2j + bh
        qw2_t = np.zeros((2, 64 * npair), np.float32)
        for kk in range(2):
            # half-quad hq covers pairs (2*hq+kk); row b = 2*(2*hq+kk)+bh
            rows = qWs[4 * np.arange(npair // 2)[:, None, None] + 2 * kk
                       + np.array([0, 1])[None, :, None],
                       np.arange(64)[None, None, :]]      # [nhq, 2, 64]
            qw2_t[kk] = rows.reshape(-1)
        # mask [128, 32*ngroups]: cols 32g + (0:16 full | 16:32 partial), 2jj+bh
        t_full = np.arange(128)[:, None]
        t_part = np.arange(128)[:, None] + 128
        mk = np.zeros((128, 32 * _ngroups), np.float32)
        for g in range(_ngroups):
            slg = sls[16 * g : 16 * g + 16]  # rows 2jj+bh in order
            mk[:, 32 * g : 32 * g + 16] = t_full < slg[None, :]
            mp = (t_part < slg[None, :]).astype(np.float32)
            mp[80:] = 0.0
            mk[:, 32 * g + 16 : 32 * g + 32] = mp
        in_maps.append({
            "key": k[b0 : b0 + nb].reshape(nb * S, D),
            "qp": qp_t.astype(np.float32),
            "qw2": qw2_t.astype(bf),
            "onehot": onehot,
            "mask": mk.astype(bf),
            "wk2": wk2, "wqk2": wqk2, "w2b": w2b,
            "w34": w34, "cols": cols,
        })

    res = run_bass_kernel_spmd(nc, in_maps, list(range(NCORES)), trace=TRACE)
    global LAST_RESULT
    LAST_RESULT = res
    outs = []
    for c in range(NCORES):
        outs.append(res.results[c]["out"])
    return np.concatenate(outs, axis=0).astype(np.float32)


# revision 35
# speedup vs baseline: 1.0221x; 1.0221x over previous
"""AttentionSequencePoolingLayer Trainium2 kernel (8-core data parallel).

B=2048, S=200, D=64, H1=64, H2=16. Batch sharded 256/core.
Dataflow per core, per group of 16 batch rows (8 pairs, 2 tokens chunks
128+80):
  kT tiles via SWDGE cast-load + xbar transpose (as before).
  qk^T per pair via DVE tensor_scalar (2x mode).
  x1 = z1 + qW accumulated fully in PSUM: rank-2 one-hot matmul folds the
       per-pair qW broadcast, so dice-1 bias/scale become pair-independent.
  p1/t1/h1 batched over 2 pairs (416 cols) to amortize per-op overhead:
       p1 = sigmoid(inv1*x1 - m1*inv1)   [ACT]
       t1 = p1*(1-a1) + a1               [DVE TS 2x]
       h1 = x1 * t1                      [DVE TT, PSUM read]
  z2 = W2^T h1 — single matmul per pair (alpha folded into t1, not W2).
  p2/t2/h2 batched over all 8 pairs (416 cols).
  scores: block-diagonal W3 -> 4 matmuls per group (token-major PSUM).
  w = sigmoid(scores)*mask; out = w^T k on PE (as before).
"""
import numpy as np
import ml_dtypes

import concourse.bacc as bacc
import concourse.tile as tile
import concourse.mybir as mybir
import concourse.bass as bass
from concourse.bass_utils import run_bass_kernel_spmd

B, S, D = 2048, 200, 64
H1, H2 = 64, 16
EPS = 1e-9
NCORES = 8
BLOC = B // NCORES          # 256 batch rows per core
NGROUPS_FULL = BLOC // 16   # 16

F32 = mybir.dt.float32
BF16 = mybir.dt.bfloat16
AF = mybir.ActivationFunctionType
ALU = mybir.AluOpType
bf = ml_dtypes.bfloat16

LP_BUFS = 6
KT_BUFS = 3
WP_BUFS = 4
PSX_BUFS = 3
PSZ_BUFS = 2
H1P_BUFS = 4
OP_BUFS = 4
GF_BUFS = None
GP_BUFS = None
GORDER = [15, 14] + list(range(14))
SC_BUFS = 1
PSP_BUFS = 2
KEY_BF16 = False

_CACHE = {}
TRACE = False
LAST_RESULT = None


def _build(ngroups, pipelined=False, t1_mode=1, po_mode=2, wt_pool=False, t2_act=False, weff_act=0, weff_pre=False, po2=False):
    nc = bacc.Bacc("TRN2", target_bir_lowering=False, debug=False, num_devices=NCORES)
    nb = 16 * ngroups           # batch rows this build processes
    npair = nb // 2

    key = nc.dram_tensor("key", [nb * S, D],
                         BF16 if KEY_BF16 else F32, kind="ExternalInput").ap()
    qp = nc.dram_tensor("qp", [128, npair], F32, kind="ExternalInput").ap()
    qw2 = nc.dram_tensor("qw2", [2, 64 * npair], BF16, kind="ExternalInput").ap()
    onehot = nc.dram_tensor("onehot", [2, 416], BF16, kind="ExternalInput").ap()
    mask = nc.dram_tensor("mask", [128, 32 * ngroups], BF16, kind="ExternalInput").ap()
    wk2 = nc.dram_tensor("wk2", [128, 128], BF16, kind="ExternalInput").ap()
    wqk2 = nc.dram_tensor("wqk2", [128, 128], BF16, kind="ExternalInput").ap()
    w2b = nc.dram_tensor("w2b", [128, 32], BF16, kind="ExternalInput").ap()
    w34 = nc.dram_tensor("w34", [128, 8], BF16, kind="ExternalInput").ap()
    cols = nc.dram_tensor("cols", [128, 8], F32, kind="ExternalInput").ap()
    # cols: 0=inv1 1=-m1*inv1 2=1-a1 3=a1 4=inv2 5=-m2*inv2 6=1-a2 7=a2
    out = nc.dram_tensor("out", [nb, D], F32, kind="ExternalOutput").ap()

    key_r = key.rearrange("(b s) d -> s b d", s=S)  # [200, nb, 64] view

    with tile.TileContext(nc) as tc:
        with (
            tc.tile_pool(name="const", bufs=1) as cp,
            tc.tile_pool(name="load", bufs=LP_BUFS) as lp,
            tc.tile_pool(name="kt", bufs=KT_BUFS) as ktp,
            tc.tile_pool(name="qk", bufs=KT_BUFS) as qkp,
            tc.tile_pool(name="work", bufs=WP_BUFS) as wp,
            tc.tile_pool(name="h1p", bufs=H1P_BUFS) as h1p,
            tc.tile_pool(name="outp", bufs=OP_BUFS) as op_,
            tc.tile_pool(name="psx", bufs=PSX_BUFS, space="PSUM") as psx,
            tc.tile_pool(name="psz", bufs=PSZ_BUFS, space="PSUM") as psz,
            tc.tile_pool(name="psp", bufs=PSP_BUFS, space="PSUM") as psp,
        ):
            # ---- constants into SBUF
            c_qp = cp.tile([128, npair], F32)
            nc.sync.dma_start(out=c_qp[:], in_=qp)
            c_qw2 = cp.tile([2, 64 * npair], BF16)
            nc.sync.dma_start(out=c_qw2[:], in_=qw2)
            c_oh = cp.tile([2, 416], BF16)
            nc.sync.dma_start(out=c_oh[:], in_=onehot)
            c_mask = cp.tile([128, 32 * ngroups], BF16)
            nc.sync.dma_start(out=c_mask[:], in_=mask)
            c_wk = cp.tile([128, 128], BF16)
            nc.sync.dma_start(out=c_wk[:], in_=wk2)
            c_wqk = cp.tile([128, 128], BF16)
            nc.sync.dma_start(out=c_wqk[:], in_=wqk2)
            c_w2 = cp.tile([128, 32], BF16)
            nc.sync.dma_start(out=c_w2[:], in_=w2b)
            c_w34 = cp.tile([128, 8], BF16)
            nc.sync.dma_start(out=c_w34[:], in_=w34)
            c_cols = cp.tile([128, 8], F32)
            nc.sync.dma_start(out=c_cols[:], in_=cols)

            st = {}   # per-group live tiles

            def stage_load(g):
                gf = lp.tile([128, 16, 64], BF16, tag="gf", name=f"gf{g}", bufs=GF_BUFS)
                nc.gpsimd.dma_start(out=gf[:], in_=key_r[0:128, 16 * g : 16 * g + 16, :])
                gp = lp.tile([128, 16, 64], BF16, tag="gp", name=f"gp{g}", bufs=GP_BUFS)
                prow = 72 if g == ngroups - 1 else 80
                if g >= ngroups - 2:
                    nc.vector.memset(gp[64:96, :, :], 0.0)
                nc.gpsimd.dma_start(
                    out=gp[0:prow, :, :],
                    in_=bass.AP(
                        key.tensor,
                        (16 * g * S + 128) * D,
                        [[D, prow], [S * D, 16], [1, D]],
                    ),
                )
                st.setdefault(g, {}).update(gf=gf, gp=gp)

            def stage_kt(g):
                s = st[g]
                gf, gp = s["gf"], s["gp"]
                ktf = ktp.tile([128, 8, 128], BF16, tag="ktf", name=f"ktf{g}")
                nc.sync.dma_start(out=ktf[:], in_=gf.rearrange("p b d -> p (b d)"),
                                  transpose=True)
                ktq = ktp.tile([128, 8, 80], BF16, tag="ktq", name=f"ktq{g}")
                nc.sync.dma_start(
                    out=ktq[:],
                    in_=gp[0:80, :, :].rearrange("p b d -> p (b d)"),
                    transpose=True,
                )
                s.update(ktf=ktf, ktq=ktq)

            def stage_weff(g):
                # wqs[:, jj, :] = diag(q_pair) @ Wqk — depends only on consts,
                # so it prefetches arbitrarily deep (off the critical chain)
                s = st[g]
                wqs = qkp.tile([128, 8, 128], BF16, tag="wqs", name=f"wqs{g}",
                               bufs=ngroups if weff_pre else None)
                for jj in range(8):
                    j = 8 * g + jj
                    if jj < weff_act:
                        nc.scalar.activation(wqs[:, jj, :], c_wqk[:], AF.Copy,
                                             scale=c_qp[:, j : j + 1])
                    else:
                        nc.vector.tensor_scalar(wqs[:, jj, :], c_wqk[:],
                                                c_qp[:, j : j + 1], None, ALU.mult)
                s["wqs"] = wqs

            def stage_a(g):
                # z1 -> x1 PSUM -> p1 -> t1 -> h1 -> z2, per half-quad (2 pairs)
                s = st[g]
                ktf, ktq, wqs = s["ktf"], s["ktq"], s["wqs"]
                s["z2"] = psz.tile([128, 416], F32, tag="z2", name=f"z2_{g}")
                h1s = []
                for m in range(4):
                    x1 = psx.tile([128, 416], F32, tag="x1", name=f"x1_{g}_{m}")
                    hq = 4 * g + m
                    nc.tensor.matmul(x1[:, 0:416],
                                     c_qw2[:, 128 * hq : 128 * hq + 128],
                                     c_oh[:], start=True, stop=False)
                    for k in range(2):
                        jj = 2 * m + k
                        base = 208 * k
                        nc.tensor.matmul(x1[:, base : base + 128], c_wk[:],
                                         ktf[:, jj, :], start=False, stop=False)
                        nc.tensor.matmul(x1[:, base : base + 128], wqs[:, jj, :],
                                         ktf[:, jj, :], start=False, stop=True)
                        nc.tensor.matmul(x1[:, base + 128 : base + 208], c_wk[:],
                                         ktq[:, jj, :], start=False, stop=False)
                        nc.tensor.matmul(x1[:, base + 128 : base + 208], wqs[:, jj, :],
                                         ktq[:, jj, :], start=False, stop=True)
                    p1 = wp.tile([128, 416], BF16, tag="p1", name=f"p1_{g}_{m}")
                    nc.scalar.activation(p1[:], x1[:], AF.Sigmoid,
                                         bias=c_cols[:, 1:2], scale=c_cols[:, 0:1])
                    t1 = wp.tile([128, 416], BF16, tag="t1", name=f"t1_{g}_{m}")
                    use_act = t1_mode == 1 or (t1_mode == 2 and m % 2 == 0)
                    if use_act:
                        nc.scalar.activation(t1[:], p1[:], AF.Identity,
                                             bias=c_cols[:, 3:4], scale=c_cols[:, 2:3])
                    else:
                        nc.vector.tensor_scalar(t1[:], p1[:], c_cols[:, 2:3],
                                                c_cols[:, 3:4], ALU.mult, ALU.add)
                    h1 = h1p.tile([128, 416], BF16, tag="h1", name=f"h1_{g}_{m}")
                    nc.vector.tensor_tensor(h1[:], x1[:], t1[:], ALU.mult)
                    for k in range(2):
                        jj = 2 * m + k
                        q4, jq = jj // 4, jj % 4
                        nc.tensor.matmul(s["z2"][32 * jq : 32 * jq + 32,
                                                 208 * q4 : 208 * q4 + 208],
                                         c_w2[:], h1[:, 208 * k : 208 * k + 208],
                                         start=True, stop=True,
                                         tile_position=(0, 32 * jq))
                    h1s.append(h1)
                s["h1s"] = h1s

            def stage_b1(g):
                # dice2 elementwise (z2 matmuls live in stage_a per half-quad)
                s = st[g]
                z2 = s["z2"]
                p2 = wp.tile([128, 416], BF16, tag="p2", name=f"p2_{g}")
                nc.scalar.activation(p2[:], z2[:], AF.Sigmoid,
                                     bias=c_cols[:, 5:6], scale=c_cols[:, 4:5])
                t2 = wp.tile([128, 416], BF16, tag="t2", name=f"t2_{g}")
                if t2_act:
                    nc.scalar.activation(t2[:], p2[:], AF.Identity,
                                         bias=c_cols[:, 7:8], scale=c_cols[:, 6:7])
                else:
                    nc.vector.tensor_scalar(t2[:], p2[:], c_cols[:, 6:7], c_cols[:, 7:8],
                                            ALU.mult, ALU.add)
                h2 = wp.tile([128, 416], BF16, tag="h2", name=f"h2_{g}")
                nc.vector.tensor_tensor(h2[:], z2[:], t2[:], ALU.mult)
                s["h2"] = h2

            def stage_b2(g):
                # scores -> sigmoid -> mask
                s = st[g]
                h2 = s["h2"]
                scores = psz.tile([128, 32], F32, tag="sc", bufs=SC_BUFS, name=f"sc{g}")
                for q4 in range(2):
                    nc.tensor.matmul(scores[0:128, 8 * q4 : 8 * q4 + 8],
                                     h2[:, 208 * q4 : 208 * q4 + 128], c_w34[:],
                                     start=True, stop=True)
                    nc.tensor.matmul(scores[0:80, 16 + 8 * q4 : 24 + 8 * q4],
                                     h2[:, 208 * q4 + 128 : 208 * q4 + 208], c_w34[:],
                                     start=True, stop=True)
                sg = wp.tile([128, 32], BF16, tag="sg", name=f"sg{g}")
                nc.scalar.activation(sg[:, 0:16], scores[:, 0:16], AF.Sigmoid)
                nc.scalar.activation(sg[0:80, 16:32], scores[0:80, 16:32], AF.Sigmoid)
                wt = wp.tile([128, 32], BF16, tag="wt", name=f"wt{g}")
                nc.vector.tensor_tensor(wt[:, 0:16], sg[:, 0:16],
                                        c_mask[:, 32 * g : 32 * g + 16], ALU.mult)
                nc.vector.tensor_tensor(wt[0:80, 16:32], sg[0:80, 16:32],
                                        c_mask[0:80, 32 * g + 16 : 32 * g + 32],
                                        ALU.mult)
                s["wt"] = wt

            def stage_b3(g):
                # pooling + output
                s = st[g]
                gf, gp, wt = s["gf"], s["gp"], s["wt"]
                if po2:
                    pool = psp.tile([128, 512], F32, tag="pool", name=f"pool{g}")
                else:
                    pool = psp.tile([128, 256], F32, tag="pool", name=f"pool{g}")
                for jw in range(8):
                    if po2:
                        pb = 32 * (jw // 4)
                        po = 128 * (jw % 4)
                    else:
                        pb = 32 * (jw // 2)
                        po = 128 * (jw % 2)
                    rhs_f = gf[:, 2 * jw : 2 * jw + 2, :].rearrange("p b d -> p (b d)")
                    rhs_p = gp[0:80, 2 * jw : 2 * jw + 2, :].rearrange("p b d -> p (b d)")
                    nc.tensor.matmul(pool[pb : pb + 2, po : po + 128],
                                     wt[0:128, 2 * jw : 2 * jw + 2], rhs_f,
                                     start=True, stop=False, tile_position=(0, pb))
                    nc.tensor.matmul(pool[pb : pb + 2, po : po + 128],
                                     wt[0:80, 16 + 2 * jw : 16 + 2 * jw + 2], rhs_p,
                                     start=False, stop=True, tile_position=(0, pb))
                if po2:
                    po_sb = op_.tile([128, 512], F32, tag="po", name=f"po{g}")
                    nc.vector.tensor_copy(po_sb[0:2, :], pool[0:2, :])
                    nc.vector.tensor_copy(po_sb[32:34, :], pool[32:34, :])
                    for bh in range(2):
                        src = po_sb[bh:64:32, :].rearrange("p (ji x) -> p ji x", x=128)
                        src = src[:, :, 64 * bh : 64 * bh + 64]
                        dst = bass.AP(out.tensor, (16 * g + bh) * D,
                                      [[8 * D, 2], [2 * D, 4], [1, D]])
                        nc.sync.dma_start(out=dst, in_=src)
                    del st[g]
                    return
                po_sb = op_.tile([128, 256], F32, tag="po", name=f"po{g}")
                if po_mode == 0:
                    eng = [nc.scalar] * 4
                elif po_mode == 1:
                    eng = [nc.scalar, nc.vector, nc.scalar, nc.vector]
                elif po_mode == 3:
                    eng = [nc.gpsimd] * 4
                elif po_mode == 4:
                    eng = [nc.vector, nc.vector, nc.vector, nc.scalar]
                elif po_mode == 5:
                    eng = [nc.scalar, nc.vector, nc.vector, nc.vector]
                else:
                    eng = [nc.vector] * 4
                for i, e in enumerate(eng):
                    r = 32 * i
                    if e is nc.scalar:
                        e.copy(po_sb[r : r + 2, :], pool[r : r + 2, :])
                    else:
                        e.tensor_copy(po_sb[r : r + 2, :], pool[r : r + 2, :])
                del st[g]
                # out rows: b = 16g + 4*bi + 2*jj + bh; sbuf row 32*bi+bh,
                # col 128*jj + 64*bh + d
                for bh in range(2):
                    src = po_sb[bh:128:32, :].rearrange("p (jj x) -> p jj x", x=128)
                    src = src[:, :, 64 * bh : 64 * bh + 64]
                    dst = bass.AP(out.tensor, (16 * g + bh) * D,
                                  [[4 * D, 4], [2 * D, 2], [1, D]])
                    nc.sync.dma_start(out=dst, in_=src)

            if pipelined:
                # software pipeline: loads 3 ahead, transposes/weff 2 ahead,
                # stage-A 1 ahead
                for gg in range(min(3, ngroups)):
                    stage_load(gg)
                for gg in range(min(2, ngroups)):
                    stage_weff(gg)
                for gg in range(min(2, ngroups)):
                    stage_kt(gg)
                stage_a(0)
                for g in range(ngroups):
                    if g + 3 < ngroups:
                        stage_load(g + 3)
                    stage_b1(g)
                    if g + 2 < ngroups:
                        stage_kt(g + 2)
                    stage_b2(g)
                    if g + 1 < ngroups:
                        stage_a(g + 1)
                    stage_b3(g)
                    if g + 2 < ngroups:
                        stage_weff(g + 2)
            else:
                if weff_pre:
                    for g in range(ngroups):
                        st[g] = {}
                        stage_weff(g)
                order = GORDER if GORDER and len(GORDER) == ngroups else range(ngroups)
                for g in order:
                    if weff_pre:
                        stage_load(g)
                    else:
                        stage_load(g)
                        stage_weff(g)
                    stage_kt(g)
                    stage_a(g)
                    stage_b1(g)
                    stage_b2(g)
                    stage_b3(g)
    nc.compile()
    return nc


def _prep_consts(W1, alpha1, mean1, var1, W2, alpha2, mean2, var2, W3):
    inv1 = 1.0 / np.sqrt(var1 + EPS)
    inv2 = 1.0 / np.sqrt(var2 + EPS)
    Wq = W1[0:64] + W1[128:192]
    Wk = W1[64:128] - W1[128:192]
    Wqk = W1[192:256]

    def blk(a):
        m = np.zeros((128, 2 * a.shape[1]), np.float32)
        m[0:64, 0 : a.shape[1]] = a
        m[64:128, a.shape[1] :] = a
        return m

    wk2 = blk(Wk).astype(bf)
    wqk2 = blk(Wqk).astype(bf)
    w2b = blk(W2).astype(bf)
    # w34: row 32*jq + 16*bh + h -> col 2*jq + bh = W3[h]
    w34 = np.zeros((128, 8), np.float32)
    for jq in range(4):
        for bh in range(2):
            w34[32 * jq + 16 * bh : 32 * jq + 16 * bh + 16, 2 * jq + bh] = W3[:, 0]
    w34 = w34.astype(bf)
    cols = np.zeros((128, 8), np.float32)
    cols[:, 0] = np.tile(inv1, 2)
    cols[:, 1] = np.tile(-mean1 * inv1, 2)
    cols[:, 2] = np.tile(1.0 - alpha1, 2)
    cols[:, 3] = np.tile(alpha1, 2)
    cols[:, 4] = np.tile(inv2, 8)
    cols[:, 5] = np.tile(-mean2 * inv2, 8)
    cols[:, 6] = np.tile(1.0 - alpha2, 8)
    cols[:, 7] = np.tile(alpha2, 8)
    onehot = np.zeros((2, 416), np.float32)
    onehot[0, 0:208] = 1.0
    onehot[1, 208:416] = 1.0
    return Wq, wk2, wqk2, w2b, w34, cols, onehot.astype(bf)


def kernel(query_emb, key_emb, seq_length, W1, alpha1, mean1, var1,
           W2, alpha2, mean2, var2, W3, _ngroups=NGROUPS_FULL):
    (Wq, wk2, wqk2, w2b, w34, cols, onehot) = _prep_consts(
        np.asarray(W1, np.float32), np.asarray(alpha1, np.float32),
        np.asarray(mean1, np.float32), np.asarray(var1, np.float32),
        np.asarray(W2, np.float32), np.asarray(alpha2, np.float32),
        np.asarray(mean2, np.float32), np.asarray(var2, np.float32),
        np.asarray(W3, np.float32))
    q = np.asarray(query_emb, np.float32)
    k = np.asarray(key_emb, np.float32)
    sl = np.asarray(seq_length).reshape(-1)

    if _ngroups not in _CACHE:
        _CACHE[_ngroups] = _build(_ngroups)
    nc = _CACHE[_ngroups]
    nb = 16 * _ngroups
    npair = nb // 2

    qW = q @ Wq  # [B, 64]

    in_maps = []
    for c in range(NCORES):
        b0 = c * BLOC
        qs = q[b0 : b0 + nb]
        qWs = qW[b0 : b0 + nb]
        sls = sl[b0 : b0 + nb]
        # pair layouts [128=(bhat,d), npair]
        qp_t = np.zeros((128, npair), np.float32)
        for bh in range(2):
            qp_t[64 * bh : 64 * bh + 64] = qs[bh::2].T
        # qw2 [2, 64*npair]: row k, cols 128*hq + 64*bh + h = qW[16g+2*(2m+k)+bh, h]
        # hq = 4g+m; pair j = 2*hq + k... j = 8g + 2m + k; batch row = 2j + bh
        qw2_t = np.zeros((2, 64 * npair), np.float32)
        for kk in range(2):
            # half-quad hq covers pairs (2*hq+kk); row b = 2*(2*hq+kk)+bh
            rows = qWs[4 * np.arange(npair // 2)[:, None, None] + 2 * kk
                       + np.array([0, 1])[None, :, None],
                       np.arange(64)[None, None, :]]      # [nhq, 2, 64]
            qw2_t[kk] = rows.reshape(-1)
        # mask [128, 32*ngroups]: cols 32g + (0:16 full | 16:32 partial), 2jj+bh
        t_full = np.arange(128)[:, None]
        t_part = np.arange(128)[:, None] + 128
        mk = np.zeros((128, 32 * _ngroups), np.float32)
        for g in range(_ngroups):
            slg = sls[16 * g : 16 * g + 16]  # rows 2jj+bh in order
            mk[:, 32 * g : 32 * g + 16] = t_full < slg[None, :]
            mp = (t_part < slg[None, :]).astype(np.float32)
            mp[80:] = 0.0
            mk[:, 32 * g + 16 : 32 * g + 32] = mp
        kshard = k[b0 : b0 + nb].reshape(nb * S, D)
        if KEY_BF16:
            kshard = kshard.astype(bf)
        in_maps.append({
            "key": kshard,
            "qp": qp_t.astype(np.float32),
            "qw2": qw2_t.astype(bf),
            "onehot": onehot,
            "mask": mk.astype(bf),
            "wk2": wk2, "wqk2": wqk2, "w2b": w2b,
            "w34": w34, "cols": cols,
        })

    res = run_bass_kernel_spmd(nc, in_maps, list(range(NCORES)), trace=TRACE)
    global LAST_RESULT
    LAST_RESULT = res
    outs = []
    for c in range(NCORES):
        outs.append(res.results[c]["out"])
    return np.concatenate(outs, axis=0).astype(np.float32)
